# revision 5
# baseline (speedup 1.0000x reference)
"""GRU layer kernel for Trainium2 (Bass/Tile), 8-core data-parallel over batch.

Problem: S=1024, B=128, I=H=256 GRU:
  x_gates = x @ W_ih^T + b_ih            (precomputed, parallel over S)
  per step: gh = h @ W_hh_t + b_hh
            r = sigmoid(xg_r + gh_r); z = sigmoid(xg_z + gh_z)
            n = tanh(xg_n + r * gh_n);  h' = (1-z)*n + z*h

Layout strategy (per core, B_loc=16):
  Everything "transposed": gate/hidden dims on SBUF partitions, batch on the
  free dim, so per-step vector/scalar ops use all 128 lanes.
  - recurrence matmul: out = gh^T chunk [128, B_loc] with W_hh_t chunks as
    stationary (lhsT = W_hh_t[k_chunk, m_chunk], already the right layout)
    and h^T [128, B_loc] moving.
  - x_gates precomputed in bulk: lhsT = W_ih^T chunks, rhs = x^T slabs.
  - h_out written back by PE-transposing windows of h^T into natural layout.
"""

import numpy as np

S, B, I, H = 1024, 128, 256, 256
G3 = 3 * H            # 768 gate dims
NCORES = 8
B_LOC = B // NCORES   # 16
NKC = H // 128        # 2 contraction chunks
NMC = G3 // 128       # 6 gate chunks (r: 0-1, z: 2-3, n: 4-5)
SB = S * B_LOC        # 16384 flattened (s, b) rows per core

# tuning knobs
XG_W = 8              # steps of x_gates fetched per DMA in recurrence
W_OUT = 64            # steps of h^T accumulated before transposing to h_out
SLAB = 512            # x_gates precompute slab width (columns of (s,b))


def _split_multi_waits(bir_json: bytes) -> bytes:
    """This container's walrus build rejects >1 sync wait per instruction
    ("Too many sync wait commands", CoreV3GenImpl.cpp setupSyncWait), which
    breaks every Tile-generated kernel at codegen. Split each instruction's
    extra waits onto same-engine NoOps placed immediately before it (engine
    queues execute in order, so the waits still gate the instruction)."""
    import json

    j = json.loads(bir_json)
    n_id = 0
    for fn in j.get("functions", []):
        for blk in fn.get("blocks", []):
            insts = blk.get("instructions", [])
            out = []
            changed = False
            for ins in insts:
                si = ins.get("sync_info")
                waits = (si or {}).get("on_wait") or []
                if len(waits) > 1:
                    changed = True
                    for w in waits[:-1]:
                        n_id += 1
                        out.append(
                            {
                                "debug": ins.get("debug", 0),
                                "engine": ins["engine"],
                                "ins": [],
                                "name": f"NW-{n_id}-{ins['name']}",
                                "opcode": "NoOp",
                                "outs": [],
                                "sync_info": {"on_update": [], "on_wait": [w]},
                            }
                        )
                    si["on_wait"] = [waits[-1]]
                out.append(ins)
            if changed:
                blk["instructions"] = out
    return json.dumps(j).encode()


_COMPILE_PATCHED = False


def _patch_compiler():
    """Install the multi-wait splitter in front of compile_bir_kernel."""
    global _COMPILE_PATCHED
    if _COMPILE_PATCHED:
        return
    import concourse.bass_utils as bass_utils
    import concourse.bass2jax as bass2jax

    orig = bass_utils.compile_bir_kernel

    def patched(bir_json, tmpdir, neff_name="file.neff"):
        if isinstance(bir_json, str):
            bir_json = bir_json.encode()
        return orig(_split_multi_waits(bir_json), tmpdir, neff_name)

    bass_utils.compile_bir_kernel = patched
    bass2jax.compile_bir_kernel = patched
    _COMPILE_PATCHED = True


def _build_bass():
    import concourse.bass as bass
    import concourse.mybir as mybir
    import concourse.tile as tile
    from concourse.masks import make_identity
    from contextlib import ExitStack

    f32 = mybir.dt.float32
    AF = mybir.ActivationFunctionType

    nc = bass.Bass()
    x_d = nc.dram_tensor("x", [S, B_LOC, I], f32, kind="ExternalInput")
    h0_d = nc.dram_tensor("h0", [B_LOC, H], f32, kind="ExternalInput")
    wih_d = nc.dram_tensor("weight_ih", [G3, I], f32, kind="ExternalInput")
    whh_d = nc.dram_tensor("weight_hh_t", [H, G3], f32, kind="ExternalInput")
    bih_d = nc.dram_tensor("bias_ih", [G3], f32, kind="ExternalInput")
    bhh_d = nc.dram_tensor("bias_hh", [G3], f32, kind="ExternalInput")
    hout_d = nc.dram_tensor("h_out", [S, B_LOC, H], f32, kind="ExternalOutput")
    xgT_d = nc.dram_tensor("xgT", [NMC, 128, SB], f32, kind="Internal")

    hout_flat = hout_d[:, :, :].flatten_outer_dims()   # [SB, H]
    x_flat = x_d[:, :, :].flatten_outer_dims()         # [SB, I]

    with ExitStack() as ctx:
        tc = ctx.enter_context(tile.TileContext(nc))
        singles = ctx.enter_context(tc.tile_pool(name="singles", bufs=1))

        identity = singles.tile([128, 128], f32)
        make_identity(nc, identity)

        # W_hh_t chunks: whh_sb[p, kc, m] = W_hh_t[kc*128+p, m]
        whh_sb = singles.tile([128, NKC, G3], f32)
        nc.sync.dma_start(
            out=whh_sb, in_=whh_d[:, :].rearrange("(kc p) m -> p kc m", p=128)
        )

        # biases as [128, 6] chunk tables
        bih_sb = singles.tile([128, NMC], f32)
        nc.sync.dma_start(
            out=bih_sb, in_=bih_d[:].rearrange("(c p) -> p c", p=128)
        )
        bhh_sb = singles.tile([128, NMC], f32)
        nc.sync.dma_start(
            out=bhh_sb, in_=bhh_d[:].rearrange("(c p) -> p c", p=128)
        )
        # per-chunk bias folded into x_gates: r/z chunks get b_ih+b_hh, n chunks b_ih
        bias_tab = singles.tile([128, NMC], f32)
        nc.vector.tensor_add(bias_tab[:, 0:4], bih_sb[:, 0:4], bhh_sb[:, 0:4])
        nc.vector.tensor_copy(bias_tab[:, 4:6], bih_sb[:, 4:6])
        # b_hh for n chunks broadcast over batch (added to gh_n before r*)
        bhhn_b = singles.tile([128, 2, B_LOC], f32)
        for c in range(2):
            nc.vector.tensor_copy(
                bhhn_b[:, c, :],
                bhh_sb[:, 4 + c : 5 + c].to_broadcast([128, B_LOC]),
            )

        # h_out accumulation windows (ping-pong)
        wins = [
            singles.tile([128, NKC, W_OUT, B_LOC], f32, name="winA"),
            singles.tile([128, NKC, W_OUT, B_LOC], f32, name="winB"),
        ]
        h_init = singles.tile([128, NKC, B_LOC], f32)

        # W_ih^T chunks: wihT[p, ic, g] = W_ih[g, ic*128+p]
        wihT = singles.tile([128, NKC, G3], f32)

        # ---------------- Phase 1: x_gates precompute ----------------
        with (
            tc.tile_pool(name="p1", bufs=3) as p1,
            tc.tile_pool(name="p1b", bufs=3) as p1b,
            tc.tile_pool(name="p1ps", bufs=2, space="PSUM") as p1ps,
            tc.tile_pool(name="p1pst", bufs=3, space="PSUM") as p1pst,
        ):
            # transpose W_ih -> wihT
            wih_sb = p1.tile([128, NMC, I], f32, tag="wih")
            nc.sync.dma_start(
                out=wih_sb, in_=wih_d[:, :].rearrange("(gc p) i -> p gc i", p=128)
            )
            for gc in range(NMC):
                for ic in range(NKC):
                    ps = p1pst.tile([128, 128], f32, tag="tps")
                    nc.tensor.transpose(
                        ps, wih_sb[:, gc, ic * 128 : (ic + 1) * 128], identity
                    )
                    nc.vector.tensor_copy(
                        wihT[:, ic, gc * 128 : (gc + 1) * 128], ps
                    )

            # transpose h0 -> h_init
            h0_sb = p1.tile([B_LOC, H], f32, tag="h0")
            nc.sync.dma_start(out=h0_sb, in_=h0_d[:, :])
            for kc in range(NKC):
                ps = p1pst.tile([128, 128], f32, tag="tps")
                nc.tensor.transpose(
                    ps[:, 0:B_LOC],
                    h0_sb[:, kc * 128 : (kc + 1) * 128],
                    identity[0:B_LOC, 0:B_LOC],
                )
                nc.vector.tensor_copy(h_init[:, kc, :], ps[:, 0:B_LOC])

            # main x_gates loop over slabs of SLAB (s,b) columns
            nslab = SB // SLAB
            nblk = SLAB // 128
            for j in range(nslab):
                xrows = p1.tile([128, nblk, I], f32, tag="xrows")
                nc.sync.dma_start(
                    out=xrows,
                    in_=x_flat[j * SLAB : (j + 1) * SLAB, :].rearrange(
                        "(t p) i -> p t i", p=128
                    ),
                )
                xT = p1.tile([128, NKC, SLAB], f32, tag="xT")
                for t in range(nblk):
                    for ic in range(NKC):
                        ps = p1pst.tile([128, 128], f32, tag="tps")
                        nc.tensor.transpose(
                            ps, xrows[:, t, ic * 128 : (ic + 1) * 128], identity
                        )
                        nc.scalar.copy(xT[:, ic, t * 128 : (t + 1) * 128], ps)
                for mc in range(NMC):
                    psxg = p1ps.tile([128, SLAB], f32, tag="psxg")
                    for ic in range(NKC):
                        nc.tensor.matmul(
                            psxg,
                            wihT[:, ic, mc * 128 : (mc + 1) * 128],
                            xT[:, ic, :],
                            start=(ic == 0),
                            stop=(ic == NKC - 1),
                        )
                    xg_sb = p1b.tile([128, SLAB], f32, tag="xg")
                    nc.vector.tensor_scalar_add(xg_sb, psxg, bias_tab[:, mc : mc + 1])
                    nc.sync.dma_start(
                        out=xgT_d[mc, :, j * SLAB : (j + 1) * SLAB], in_=xg_sb
                    )

        # ---------------- Phase 2: recurrence ----------------
        with (
            tc.tile_pool(name="p2w", bufs=2) as p2w,
            tc.tile_pool(name="p2s", bufs=3) as p2s,
            tc.tile_pool(name="p2o", bufs=3) as p2o,
            tc.tile_pool(name="p2rz", bufs=2, space="PSUM") as p2rz,
            tc.tile_pool(name="p2n", bufs=2, space="PSUM") as p2n,
            tc.tile_pool(name="p2t", bufs=3, space="PSUM") as p2t,
        ):
            xg_win = None
            for s in range(S):
                w_idx = (s // W_OUT) % 2
                s_loc = s % W_OUT
                if s % XG_W == 0:
                    xg_win = p2w.tile([128, NMC, XG_W, B_LOC], f32, tag="xgw")
                    nc.sync.dma_start(
                        out=xg_win,
                        in_=xgT_d[:, :, s * B_LOC : (s + XG_W) * B_LOC].rearrange(
                            "c p (t b) -> p c t b", b=B_LOC
                        ),
                    )
                t_loc = s % XG_W

                if s == 0:
                    hT_prev = h_init[:, :, :]
                else:
                    hT_prev = wins[((s - 1) // W_OUT) % 2][:, :, (s - 1) % W_OUT, :]

                ps_rz = p2rz.tile([128, 4, B_LOC], f32, tag="psrz")
                ps_n = p2n.tile([128, 2, B_LOC], f32, tag="psn")
                for mc in range(4):
                    for kc in range(NKC):
                        nc.tensor.matmul(
                            ps_rz[:, mc, :],
                            whh_sb[:, kc, mc * 128 : (mc + 1) * 128],
                            hT_prev[:, kc, :],
                            start=(mc == 0 and kc == 0),
                            stop=(mc == 3 and kc == NKC - 1),
                        )
                for mc in range(2):
                    for kc in range(NKC):
                        nc.tensor.matmul(
                            ps_n[:, mc, :],
                            whh_sb[:, kc, (mc + 4) * 128 : (mc + 5) * 128],
                            hT_prev[:, kc, :],
                            start=(mc == 0 and kc == 0),
                            stop=(mc == 1 and kc == NKC - 1),
                        )

                rzpre = p2s.tile([128, 4, B_LOC], f32, tag="rzpre")
                nc.vector.tensor_add(rzpre, ps_rz, xg_win[:, 0:4, t_loc, :])
                rz = p2s.tile([128, 4, B_LOC], f32, tag="rz")
                nc.scalar.activation(rz, rzpre, AF.Sigmoid)

                ghn = p2s.tile([128, 2, B_LOC], f32, tag="ghn")
                nc.vector.tensor_add(ghn, ps_n, bhhn_b)
                a_t = p2s.tile([128, 2, B_LOC], f32, tag="a")
                nc.vector.tensor_mul(a_t, rz[:, 0:2, :], ghn)
                npre = p2s.tile([128, 2, B_LOC], f32, tag="npre")
                nc.vector.tensor_add(npre, a_t, xg_win[:, 4:6, t_loc, :])
                n_t = p2s.tile([128, 2, B_LOC], f32, tag="n")
                nc.scalar.activation(n_t, npre, AF.Tanh)

                d_t = p2s.tile([128, 2, B_LOC], f32, tag="d")
                nc.vector.tensor_sub(d_t, hT_prev, n_t)
                e_t = p2s.tile([128, 2, B_LOC], f32, tag="e")
                nc.vector.tensor_mul(e_t, rz[:, 2:4, :], d_t)
                nc.vector.tensor_add(wins[w_idx][:, :, s_loc, :], n_t, e_t)

                # flush a completed h^T window to h_out in natural layout
                if s_loc == W_OUT - 1:
                    win = wins[w_idx]
                    base = (s - W_OUT + 1) * B_LOC
                    for t in range(W_OUT * B_LOC // 128):
                        hnat = p2o.tile([128, NKC, 128], f32, tag="hnat")
                        for kc in range(NKC):
                            pst = p2t.tile([128, 128], f32, tag="hps")
                            nc.tensor.transpose(
                                pst, win[:, kc, t * 8 : (t + 1) * 8, :], identity
                            )
                            if kc == 0:
                                nc.scalar.copy(hnat[:, kc, :], pst)
                            else:
                                nc.vector.tensor_copy(hnat[:, kc, :], pst)
                        nc.sync.dma_start(
                            out=hout_flat[base + t * 128 : base + (t + 1) * 128, :],
                            in_=hnat,
                        )

    return nc


_NC = None
_LAST_RESULT = None


def _get_nc():
    global _NC
    if _NC is None:
        _NC = _build_bass()
    return _NC


def kernel(x, h0, weight_ih, weight_hh_t, bias_ih, bias_hh):
    _patch_compiler()
    from concourse.bass_utils import run_bass_kernel_spmd

    x = np.ascontiguousarray(np.asarray(x, dtype=np.float32))
    h0 = np.ascontiguousarray(np.asarray(h0, dtype=np.float32))
    weight_ih = np.ascontiguousarray(np.asarray(weight_ih, dtype=np.float32))
    weight_hh_t = np.ascontiguousarray(np.asarray(weight_hh_t, dtype=np.float32))
    bias_ih = np.ascontiguousarray(np.asarray(bias_ih, dtype=np.float32))
    bias_hh = np.ascontiguousarray(np.asarray(bias_hh, dtype=np.float32))

    nc = _get_nc()
    in_maps = []
    for c in range(NCORES):
        bsl = slice(c * B_LOC, (c + 1) * B_LOC)
        in_maps.append(
            {
                "x": np.ascontiguousarray(x[:, bsl, :]),
                "h0": np.ascontiguousarray(h0[bsl, :]),
                "weight_ih": weight_ih,
                "weight_hh_t": weight_hh_t,
                "bias_ih": bias_ih,
                "bias_hh": bias_hh,
            }
        )
    res = run_bass_kernel_spmd(nc, in_maps, core_ids=list(range(NCORES)))
    global _LAST_RESULT
    _LAST_RESULT = res
    h_out = np.concatenate([r["h_out"] for r in res.results], axis=1)
    h_final = np.ascontiguousarray(h_out[-1])
    return h_out, h_final


# revision 12
# speedup vs baseline: 2.4920x; 2.4920x over previous
"""GRU layer kernel for Trainium2 (Bass/Tile), 8-core data-parallel over batch.

Problem: S=1024, B=128, I=H=256 GRU:
  x_gates = x @ W_ih^T + b_ih            (precomputed, parallel over S)
  per step: gh = h @ W_hh_t + b_hh
            r = sigmoid(xg_r + gh_r); z = sigmoid(xg_z + gh_z)
            n = tanh(xg_n + r * gh_n);  h' = (1-z)*n + z*h

Layout strategy (per core, B_loc=16):
  Everything "transposed": gate/hidden dims on SBUF partitions, batch on the
  free dim, so per-step vector/scalar ops use all 128 lanes.
  - recurrence matmul: out = gh^T chunk [128, B_loc] with W_hh_t chunks as
    stationary (lhsT = W_hh_t[k_chunk, m_chunk], already the right layout)
    and h^T [128, B_loc] moving.
  - x_gates precomputed in bulk: lhsT = W_ih^T chunks, rhs = x^T slabs.
  - h_out written back by PE-transposing windows of h^T into natural layout.
"""

import numpy as np

S, B, I, H = 1024, 128, 256, 256
G3 = 3 * H            # 768 gate dims
NCORES = 8
B_LOC = B // NCORES   # 16
NKC = H // 128        # 2 contraction chunks
NMC = G3 // 128       # 6 gate chunks (r: 0-1, z: 2-3, n: 4-5)
SB = S * B_LOC        # 16384 flattened (s, b) rows per core

# tuning knobs
XG_W = 8              # steps of x_gates fetched per DMA in recurrence
W_OUT = 64            # steps of h^T accumulated before transposing to h_out
SLAB = 512            # x_gates precompute slab width (columns of (s,b))


def _split_multi_waits(bir_json: bytes) -> bytes:
    """This container's walrus build rejects >1 sync wait per instruction
    ("Too many sync wait commands", CoreV3GenImpl.cpp setupSyncWait), which
    breaks every Tile-generated kernel at codegen. Split each instruction's
    extra waits onto same-engine NoOps placed immediately before it (engine
    queues execute in order, so the waits still gate the instruction)."""
    import json

    j = json.loads(bir_json)
    n_id = 0
    for fn in j.get("functions", []):
        for blk in fn.get("blocks", []):
            insts = blk.get("instructions", [])
            out = []
            changed = False
            for ins in insts:
                si = ins.get("sync_info")
                waits = (si or {}).get("on_wait") or []
                if len(waits) > 1:
                    changed = True
                    for w in waits[:-1]:
                        n_id += 1
                        out.append(
                            {
                                "debug": ins.get("debug", 0),
                                "engine": ins["engine"],
                                "ins": [],
                                "name": f"NW-{n_id}-{ins['name']}",
                                "opcode": "NoOp",
                                "outs": [],
                                "sync_info": {"on_update": [], "on_wait": [w]},
                            }
                        )
                    si["on_wait"] = [waits[-1]]
                out.append(ins)
            if changed:
                blk["instructions"] = out
    return json.dumps(j).encode()


_COMPILE_PATCHED = False


def _patch_compiler():
    """Install the multi-wait splitter in front of compile_bir_kernel."""
    global _COMPILE_PATCHED
    if _COMPILE_PATCHED:
        return
    import concourse.bass_utils as bass_utils
    import concourse.bass2jax as bass2jax

    orig = bass_utils.compile_bir_kernel

    def patched(bir_json, tmpdir, neff_name="file.neff"):
        if isinstance(bir_json, str):
            bir_json = bir_json.encode()
        return orig(_split_multi_waits(bir_json), tmpdir, neff_name)

    bass_utils.compile_bir_kernel = patched
    bass2jax.compile_bir_kernel = patched
    _COMPILE_PATCHED = True


def _build_bass():
    import concourse.bass as bass
    import concourse.mybir as mybir
    import concourse.tile as tile
    from concourse.masks import make_identity
    from contextlib import ExitStack

    f32 = mybir.dt.float32
    bf16 = mybir.dt.bfloat16
    AF = mybir.ActivationFunctionType

    nc = bass.Bass()
    x_d = nc.dram_tensor("x", [S, B_LOC, I], f32, kind="ExternalInput")
    h0_d = nc.dram_tensor("h0", [B_LOC, H], f32, kind="ExternalInput")
    wih_d = nc.dram_tensor("weight_ih", [G3, I], f32, kind="ExternalInput")
    whh_d = nc.dram_tensor("weight_hh_t", [H, G3], f32, kind="ExternalInput")
    bih_d = nc.dram_tensor("bias_ih", [G3], f32, kind="ExternalInput")
    bhh_d = nc.dram_tensor("bias_hh", [G3], f32, kind="ExternalInput")
    hout_d = nc.dram_tensor("h_out", [S, B_LOC, H], f32, kind="ExternalOutput")
    xgT_d = nc.dram_tensor("xgT", [NMC, 128, SB], f32, kind="Internal")

    hout_flat = hout_d[:, :, :].flatten_outer_dims()   # [SB, H]
    x_flat = x_d[:, :, :].flatten_outer_dims()         # [SB, I]

    with ExitStack() as ctx:
        tc = ctx.enter_context(tile.TileContext(nc))
        singles = ctx.enter_context(tc.tile_pool(name="singles", bufs=1))

        identity = singles.tile([128, 128], f32)
        make_identity(nc, identity)
        identity_bf = singles.tile([128, 128], bf16)
        nc.vector.tensor_copy(identity_bf, identity)

        # W_hh_t chunks (bf16): whh_sb[p, kc, m] = W_hh_t[kc*128+p, m]
        whh_f32 = singles.tile([128, NKC, G3], f32)
        nc.sync.dma_start(
            out=whh_f32, in_=whh_d[:, :].rearrange("(kc p) m -> p kc m", p=128)
        )
        whh_sb = singles.tile([128, NKC, G3], bf16)
        nc.vector.tensor_copy(whh_sb, whh_f32)

        # biases as [128, 6] chunk tables
        bih_sb = singles.tile([128, NMC], f32)
        nc.sync.dma_start(
            out=bih_sb, in_=bih_d[:].rearrange("(c p) -> p c", p=128)
        )
        bhh_sb = singles.tile([128, NMC], f32)
        nc.sync.dma_start(
            out=bhh_sb, in_=bhh_d[:].rearrange("(c p) -> p c", p=128)
        )
        # per-chunk bias folded into x_gates: r/z chunks get b_ih+b_hh, n chunks b_ih
        bias_tab = singles.tile([128, NMC], f32)
        nc.vector.tensor_add(bias_tab[:, 0:4], bih_sb[:, 0:4], bhh_sb[:, 0:4])
        nc.vector.tensor_copy(bias_tab[:, 4:6], bih_sb[:, 4:6])
        # b_hh for n chunks broadcast over batch (added to gh_n before r*)
        bhhn_b = singles.tile([128, 2, B_LOC], f32)
        for c in range(2):
            nc.vector.tensor_copy(
                bhhn_b[:, c, :],
                bhh_sb[:, 4 + c : 5 + c].to_broadcast([128, B_LOC]),
            )

        # h_out accumulation windows (ping-pong), bf16 recurrent state
        wins = [
            singles.tile([128, NKC, W_OUT, B_LOC], bf16, name="winA"),
            singles.tile([128, NKC, W_OUT, B_LOC], bf16, name="winB"),
        ]
        h_init = singles.tile([128, NKC, B_LOC], bf16)

        # W_ih^T chunks (bf16): wihT[p, ic, g] = W_ih[g, ic*128+p]
        wihT = singles.tile([128, NKC, G3], bf16)

        # ---------------- Phase 1: x_gates precompute ----------------
        with (
            tc.tile_pool(name="p1", bufs=3) as p1,
            tc.tile_pool(name="p1b", bufs=3) as p1b,
            tc.tile_pool(name="p1ps", bufs=2, space="PSUM") as p1ps,
            tc.tile_pool(name="p1pst", bufs=3, space="PSUM") as p1pst,
        ):
            # transpose W_ih -> wihT (cast to bf16 first)
            wih_f32 = p1.tile([128, NMC, I], f32, tag="wih")
            nc.sync.dma_start(
                out=wih_f32, in_=wih_d[:, :].rearrange("(gc p) i -> p gc i", p=128)
            )
            wih_sb = p1.tile([128, NMC, I], bf16, tag="wihb")
            nc.vector.tensor_copy(wih_sb, wih_f32)
            for gc in range(NMC):
                for ic in range(NKC):
                    ps = p1pst.tile([128, 128], bf16, tag="tpsb")
                    nc.tensor.transpose(
                        ps, wih_sb[:, gc, ic * 128 : (ic + 1) * 128], identity_bf
                    )
                    nc.vector.tensor_copy(
                        wihT[:, ic, gc * 128 : (gc + 1) * 128], ps
                    )

            # transpose h0 -> h_init
            h0_sb = p1.tile([B_LOC, H], f32, tag="h0")
            nc.sync.dma_start(out=h0_sb, in_=h0_d[:, :])
            for kc in range(NKC):
                ps = p1pst.tile([128, 128], f32, tag="tps")
                nc.tensor.transpose(
                    ps[:, 0:B_LOC],
                    h0_sb[:, kc * 128 : (kc + 1) * 128],
                    identity[0:B_LOC, 0:B_LOC],
                )
                nc.vector.tensor_copy(h_init[:, kc, :], ps[:, 0:B_LOC])

            # main x_gates loop over slabs of SLAB (s,b) columns
            nslab = SB // SLAB
            nblk = SLAB // 128
            for j in range(nslab):
                xrows_f = p1.tile([128, nblk, I], f32, tag="xrows")
                nc.sync.dma_start(
                    out=xrows_f,
                    in_=x_flat[j * SLAB : (j + 1) * SLAB, :].rearrange(
                        "(t p) i -> p t i", p=128
                    ),
                )
                xrows = p1.tile([128, nblk, I], bf16, tag="xrowsb")
                nc.vector.tensor_copy(xrows, xrows_f)
                xT = p1.tile([128, NKC, SLAB], bf16, tag="xT")
                for t in range(nblk):
                    for ic in range(NKC):
                        ps = p1pst.tile([128, 128], bf16, tag="tpsb")
                        nc.tensor.transpose(
                            ps, xrows[:, t, ic * 128 : (ic + 1) * 128], identity_bf
                        )
                        nc.scalar.copy(xT[:, ic, t * 128 : (t + 1) * 128], ps)
                for mc in range(NMC):
                    psxg = p1ps.tile([128, SLAB], f32, tag="psxg")
                    for ic in range(NKC):
                        nc.tensor.matmul(
                            psxg,
                            wihT[:, ic, mc * 128 : (mc + 1) * 128],
                            xT[:, ic, :],
                            start=(ic == 0),
                            stop=(ic == NKC - 1),
                        )
                    xg_sb = p1b.tile([128, SLAB], f32, tag="xg")
                    nc.vector.tensor_scalar_add(xg_sb, psxg, bias_tab[:, mc : mc + 1])
                    nc.sync.dma_start(
                        out=xgT_d[mc, :, j * SLAB : (j + 1) * SLAB], in_=xg_sb
                    )

        # ---------------- Phase 2: recurrence ----------------
        with (
            tc.tile_pool(name="p2w", bufs=2) as p2w,
            tc.tile_pool(name="p2s", bufs=3) as p2s,
            tc.tile_pool(name="p2o", bufs=3) as p2o,
            tc.tile_pool(name="p2rz", bufs=2, space="PSUM") as p2rz,
            tc.tile_pool(name="p2n", bufs=2, space="PSUM") as p2n,
            tc.tile_pool(name="p2t", bufs=3, space="PSUM") as p2t,
        ):
            xg_win = None
            for s in range(S):
                w_idx = (s // W_OUT) % 2
                s_loc = s % W_OUT
                if s % XG_W == 0:
                    xg_win = p2w.tile([128, NMC, XG_W, B_LOC], f32, tag="xgw")
                    nc.sync.dma_start(
                        out=xg_win,
                        in_=xgT_d[:, :, s * B_LOC : (s + XG_W) * B_LOC].rearrange(
                            "c p (t b) -> p c t b", b=B_LOC
                        ),
                    )
                t_loc = s % XG_W

                if s == 0:
                    hT_prev = h_init[:, :, :]
                else:
                    hT_prev = wins[((s - 1) // W_OUT) % 2][:, :, (s - 1) % W_OUT, :]

                ps_rz = p2rz.tile([128, 4, B_LOC], f32, tag="psrz")
                ps_n = p2n.tile([128, 2, B_LOC], f32, tag="psn")
                for mc in range(4):
                    for kc in range(NKC):
                        nc.tensor.matmul(
                            ps_rz[:, mc, :],
                            whh_sb[:, kc, mc * 128 : (mc + 1) * 128],
                            hT_prev[:, kc, :],
                            start=(mc == 0 and kc == 0),
                            stop=(mc == 3 and kc == NKC - 1),
                        )
                for mc in range(2):
                    for kc in range(NKC):
                        nc.tensor.matmul(
                            ps_n[:, mc, :],
                            whh_sb[:, kc, (mc + 4) * 128 : (mc + 5) * 128],
                            hT_prev[:, kc, :],
                            start=(mc == 0 and kc == 0),
                            stop=(mc == 1 and kc == NKC - 1),
                        )

                rzpre = p2s.tile([128, 4, B_LOC], f32, tag="rzpre")
                nc.vector.tensor_add(rzpre, ps_rz, xg_win[:, 0:4, t_loc, :])
                rz = p2s.tile([128, 4, B_LOC], f32, tag="rz")
                nc.scalar.activation(rz, rzpre, AF.Sigmoid)

                ghn = p2s.tile([128, 2, B_LOC], f32, tag="ghn")
                nc.vector.tensor_add(ghn, ps_n, bhhn_b)
                a_t = p2s.tile([128, 2, B_LOC], f32, tag="a")
                nc.vector.tensor_mul(a_t, rz[:, 0:2, :], ghn)
                npre = p2s.tile([128, 2, B_LOC], f32, tag="npre")
                nc.vector.tensor_add(npre, a_t, xg_win[:, 4:6, t_loc, :])
                n_t = p2s.tile([128, 2, B_LOC], f32, tag="n")
                nc.scalar.activation(n_t, npre, AF.Tanh)

                d_t = p2s.tile([128, 2, B_LOC], f32, tag="d")
                nc.vector.tensor_sub(d_t, hT_prev, n_t)
                e_t = p2s.tile([128, 2, B_LOC], f32, tag="e")
                nc.vector.tensor_mul(e_t, rz[:, 2:4, :], d_t)
                nc.vector.tensor_add(wins[w_idx][:, :, s_loc, :], n_t, e_t)

                # flush a completed h^T window to h_out in natural layout
                if s_loc == W_OUT - 1:
                    win = wins[w_idx]
                    base = (s - W_OUT + 1) * B_LOC
                    for t in range(W_OUT * B_LOC // 128):
                        hnat = p2o.tile([128, NKC, 128], f32, tag="hnat")
                        for kc in range(NKC):
                            pst = p2t.tile([128, 128], bf16, tag="hps")
                            nc.tensor.transpose(
                                pst, win[:, kc, t * 8 : (t + 1) * 8, :], identity_bf
                            )
                            if kc == 0:
                                nc.scalar.copy(hnat[:, kc, :], pst)
                            else:
                                nc.vector.tensor_copy(hnat[:, kc, :], pst)
                        nc.sync.dma_start(
                            out=hout_flat[base + t * 128 : base + (t + 1) * 128, :],
                            in_=hnat,
                        )

    return nc


_NC = None
_LAST_RESULT = None


def _get_nc():
    global _NC
    if _NC is None:
        _NC = _build_bass()
    return _NC


def kernel(x, h0, weight_ih, weight_hh_t, bias_ih, bias_hh):
    _patch_compiler()
    from concourse.bass_utils import run_bass_kernel_spmd

    x = np.ascontiguousarray(np.asarray(x, dtype=np.float32))
    h0 = np.ascontiguousarray(np.asarray(h0, dtype=np.float32))
    weight_ih = np.ascontiguousarray(np.asarray(weight_ih, dtype=np.float32))
    weight_hh_t = np.ascontiguousarray(np.asarray(weight_hh_t, dtype=np.float32))
    bias_ih = np.ascontiguousarray(np.asarray(bias_ih, dtype=np.float32))
    bias_hh = np.ascontiguousarray(np.asarray(bias_hh, dtype=np.float32))

    nc = _get_nc()
    in_maps = []
    for c in range(NCORES):
        bsl = slice(c * B_LOC, (c + 1) * B_LOC)
        in_maps.append(
            {
                "x": np.ascontiguousarray(x[:, bsl, :]),
                "h0": np.ascontiguousarray(h0[bsl, :]),
                "weight_ih": weight_ih,
                "weight_hh_t": weight_hh_t,
                "bias_ih": bias_ih,
                "bias_hh": bias_hh,
            }
        )
    res = run_bass_kernel_spmd(nc, in_maps, core_ids=list(range(NCORES)))
    global _LAST_RESULT
    _LAST_RESULT = res
    h_out = np.concatenate([r["h_out"] for r in res.results], axis=1)
    h_final = np.ascontiguousarray(h_out[-1])
    return h_out, h_final
